# revision 38
# baseline (speedup 1.0000x reference)
"""Distributed causal multi-head attention for one TRN2 chip (8 NeuronCores).

Sharding: batch (2) x head-groups (4 heads/core) -> 8 cores.
Core c handles batch c//4, heads [ (c%4)*4 , (c%4)*4+4 ).
Per core: QKV projections for its 4 heads, flash-style causal attention
with scores kept transposed (S^T = K @ Q^T) so the PV product needs no
transposes; V is augmented with a ones column so the softmax denominators
fall out of the same matmul (row 64 of each head's O^T psum).  Then an
AllGather of the attention output (pre-Wo, 4-core group = one batch) and
a column-sliced output projection.  Host assembles the 8 column/batch
shards.  Compute dtype bf16 (PSUM accumulation fp32), softmax in fp32.

Scheduling notes:
- All host inputs are repacked partition-major so every input tensor
  loads with a handful of large contiguous DMAs.
- The attention pipeline is ACT(exp)-rate-limited; KT/V/Q projections
  for later chunks and AllGather-gated output projections are queued
  work items drained into the PE's idle slots.
- AllGather-gated DMA loads ride the sync queue only: at the head of
  the scalar/gpsimd queue they would block exp/broadcast behind them.
- Projections are drained lazily (reservoir) so PE work remains to fill
  AllGather flight windows; the flight of the final half-chunk gather
  is covered by reserved projections plus the pair-A halves of the last
  chunk's projections (held psum accumulation), keeping the HAM clock
  gate warm through the tail.
"""

import sys
from collections import deque

import numpy as np

sys.path.insert(0, "/opt/trn_rl_repo")

import concourse.bass as bass  # noqa: E402
import concourse.bacc as bacc  # noqa: E402
import concourse.tile as tile  # noqa: E402
import concourse.mybir as mybir  # noqa: E402

F32 = mybir.dt.float32
BF16 = mybir.dt.bfloat16
ActFn = mybir.ActivationFunctionType

P = 128          # partition dim
CHUNK = 512      # i-chunk (matmul moving free dim, one psum bank of fp32)
DH = 64          # head dim
HPC = 4          # heads per core
HS = HPC * DH    # 256 per-core inner slice
DHA = DH + 1     # augmented head dim (ones column for softmax sums)
INNER = 1024     # total inner dim (16 heads x 64)
N_CORES = 8
GROUPS = [[0, 1, 2, 3], [4, 5, 6, 7]]


def build_nc(seq=2048, dim=1024, n_cores=N_CORES, groups=GROUPS, compile=True):
    """Build the SPMD Bass graph (identical on all cores)."""
    nch = seq // CHUNK          # i-chunks
    jpc = CHUNK // P            # j-tiles per chunk (4)
    njt = seq // P              # j-tiles
    nk = dim // P               # feature k-tiles
    nko = INNER // P            # inner k-tiles for the output projection
    grp = len(groups[0])        # replica group size (4)

    nc = bacc.Bacc("TRN2", target_bir_lowering=False, debug=False,
                   enable_asserts=False, num_devices=n_cores)

    # all inputs are host-repacked partition-major: [P, ntiles*cols]
    xT = nc.dram_tensor("xT", [P, nk * seq], BF16, kind="ExternalInput").ap()
    wq = nc.dram_tensor("wq", [P, nk * HS], BF16, kind="ExternalInput").ap()
    wk = nc.dram_tensor("wk", [P, nk * HS], BF16, kind="ExternalInput").ap()
    wv = nc.dram_tensor("wv", [P, nk * HS], BF16, kind="ExternalInput").ap()
    wo = nc.dram_tensor("wo", [P, nko * HS], BF16, kind="ExternalInput").ap()
    mask_c = nc.dram_tensor("mask_c", [P, P], BF16, kind="ExternalInput").ap()
    outT = nc.dram_tensor("outT", [HS, seq], BF16, kind="ExternalOutput").ap()

    with tile.TileContext(nc) as tc:
        with tc.tile_pool(name="sb", bufs=1) as sb, \
             tc.tile_pool(name="ps", bufs=1, space="PSUM") as ps, \
             tc.tile_pool(name="dram", bufs=1, space="DRAM") as dram:

            dma_engines = [nc.sync, nc.scalar, nc.gpsimd]

            # ---- input loads: few, large, contiguous DMAs; xt halves
            # k-grouped across the three DMA queues so chunk-0 columns of
            # every k-tile land within the first few microseconds ----
            xts = sb.tile([P, nk * seq], BF16, tag="xts", name="xts")
            xt = [xts[:, k * seq:(k + 1) * seq] for k in range(nk)]
            wqs = sb.tile([P, nk * HS], BF16, tag="wqs", name="wqs")
            wq_sb = [wqs[:, k * HS:(k + 1) * HS] for k in range(nk)]
            wks = sb.tile([P, nk * HS], BF16, tag="wks", name="wks")
            wk_sb = [wks[:, k * HS:(k + 1) * HS] for k in range(nk)]
            wvs = sb.tile([P, nk * HS], BF16, tag="wvs", name="wvs")
            wv_sb = [wvs[:, k * HS:(k + 1) * HS] for k in range(nk)]
            wos = sb.tile([P, nko * HS], BF16, tag="wos", name="wos")
            wo_sb = [wos[:, k * HS:(k + 1) * HS] for k in range(nko)]
            mask_sb = sb.tile([P, P], BF16, tag="mask", name="mask")

            # half-width per-k loads: 2KB contiguous runs (full DMA rate),
            # per-tile semaphores so the QKV k-loops stream as tiles land
            half = seq // 2
            wh = nk * HS // 2
            nc.gpsimd.dma_start(mask_sb[:], mask_c[:])
            nc.sync.dma_start(wqs[:, 0:wh], wq[:, 0:wh])
            nc.scalar.dma_start(wqs[:, wh:], wq[:, wh:])
            for k in range(nk):
                dma_engines[k % 3].dma_start(
                    xt[k][:, 0:half], xT[:, k * seq:k * seq + half])
                if k == min(2, nk - 1):
                    nc.sync.dma_start(wks[:, 0:wh], wk[:, 0:wh])
                    nc.scalar.dma_start(wks[:, wh:], wk[:, wh:])
                    nc.gpsimd.dma_start(wvs[:], wv[:])
            for k in range(nk):
                dma_engines[k % 3].dma_start(
                    xt[k][:, half:seq], xT[:, k * seq + half:(k + 1) * seq])
            nc.scalar.dma_start(wos[:], wo[:])

            # warm up the collectives firmware (tiny: the entry barrier's
            # length varies run to run, so a real-sized warmup can land in
            # the critical CC window).  Emitted after the input loads: its
            # trigger waits on the warm_in DMA and would otherwise block
            # the gpsimd queue's share of the input loads.
            warm_in = dram.tile([P, 4], BF16, tag="warm_i", name="warm_i")
            warm_out = dram.tile([grp * P, 4], BF16,
                                 tag="warm_o", name="warm_o")
            nc.gpsimd.dma_start(warm_in[:], xT[0:P, 0:4])
            nc.gpsimd.collective_compute(
                "AllGather", mybir.AluOpType.bypass, replica_groups=groups,
                ins=[warm_in.opt()], outs=[warm_out.opt()])

            # persistent QKV results
            qt_sb = [sb.tile([P, seq], BF16, tag=f"qt{p}", name=f"qt{p}")
                     for p in range(2)]
            kt_sb = [sb.tile([P, seq], BF16, tag=f"kt{p}", name=f"kt{p}")
                     for p in range(2)]
            v_sb = [sb.tile([P, HPC * DHA], BF16, tag=f"v{j}", name=f"v{j}")
                    for j in range(njt)]
            ot_sb = [sb.tile([P, seq], BF16, tag=f"ot{p}", name=f"ot{p}")
                     for p in range(2)]

            # ---- interleavable work items (each emits one psum group) ----
            def emit_kt(pair, ch):
                pt = ps.tile([P, CHUNK], F32, tag="misc",
                             name=f"ktps{pair}_{ch}", bufs=2)
                for k in range(nk):
                    nc.tensor.matmul(
                        pt[:], lhsT=wk_sb[k][:, pair * P:(pair + 1) * P],
                        rhs=xt[k][:, ch * CHUNK:(ch + 1) * CHUNK],
                        start=(k == 0), stop=(k == nk - 1))
                nc.vector.tensor_copy(
                    kt_sb[pair][:, ch * CHUNK:(ch + 1) * CHUNK], pt[:])

            def emit_v(jt):
                pt = ps.tile([P, HS], F32, tag="misc",
                             name=f"vps{jt}", bufs=2)
                for k in range(nk):
                    nc.tensor.matmul(
                        pt[:], lhsT=xt[k][:, jt * P:(jt + 1) * P],
                        rhs=wv_sb[k][:],
                        start=(k == 0), stop=(k == nk - 1))
                nc.vector.tensor_copy(
                    v_sb[jt].rearrange("p (h d) -> p h d", h=HPC)[:, :, 0:DH],
                    pt.rearrange("p (h d) -> p h d", h=HPC))
                nc.vector.memset(
                    v_sb[jt].rearrange("p (h d) -> p h d", h=HPC)[:, :, DH:DHA],
                    1.0)

            def emit_qt(pair, ch):
                pt = ps.tile([P, CHUNK], F32, tag="misc",
                             name=f"qps{pair}_{ch}", bufs=2)
                for k in range(nk):
                    nc.tensor.matmul(
                        pt[:],
                        lhsT=wq_sb[k][:, pair * P:(pair + 1) * P],
                        rhs=xt[k][:, ch * CHUNK:(ch + 1) * CHUNK],
                        start=(k == 0), stop=(k == nk - 1))
                nc.vector.tensor_copy(
                    qt_sb[pair][:, ch * CHUNK:(ch + 1) * CHUNK], pt[:])

            def emit_proj(ci, m, slices, op_ps=None, evac=True,
                          lo=0, hi=CHUNK, stop=None):
                # transposed output block: outT[m*128:(m+1)*128, chunk ci]
                # = Wo[:, m-slice].T @ attT[:, chunk] over the k-tiles in
                # `slices`, columns [lo,hi) (partial passes keep op_ps
                # alive; has_written bits track sub-regions)
                c0 = ci * CHUNK
                first = op_ps is None
                if first:
                    op_ps = ps.tile([P, CHUNK], F32, tag="misc",
                                    name=f"op{ci}_{m}", bufs=2)
                if stop is None:
                    stop = evac
                for n, (k, ag_t, coff) in enumerate(slices):
                    nc.tensor.matmul(
                        op_ps[:, lo:hi],
                        lhsT=wo_sb[k][:, m * P:(m + 1) * P],
                        rhs=ag_t[:, coff + lo:coff + hi],
                        start=(first and n == 0),
                        stop=(stop and n == len(slices) - 1))
                if not evac:
                    return op_ps
                o_sb = sb.tile([P, CHUNK], BF16, tag="osb",
                               name=f"o{ci}_{m}", bufs=4)
                nc.vector.tensor_copy(o_sb[:], op_ps[:])
                nc.sync.dma_start(outT[m * P:(m + 1) * P, c0:c0 + CHUNK],
                                  o_sb[:])
                return None

            work_early = deque()   # KT/V/Q for future chunks (not gated)
            work_late = deque()    # output projections (gated on AllGather)

            def pop_work(late_ok, late_floor):
                if work_early:
                    work_early.popleft()()
                elif late_ok and len(work_late) > late_floor:
                    work_late.popleft()()

            def emit_ag_full(ci, bounce_in):
                # one AllGather for both head pairs of chunk `ci` (256KB —
                # amortizes the ncfw floor; rank-major rows land so that
                # gathered row-block k*128 is exactly attT k-tile k)
                bounce_out = dram.tile([grp * 2 * P, CHUNK], BF16,
                                       tag="boutf", name=f"boutf{ci}", bufs=2)
                nc.gpsimd.collective_compute(
                    "AllGather", mybir.AluOpType.bypass,
                    replica_groups=groups,
                    ins=[bounce_in.opt()], outs=[bounce_out.opt()])
                tiles = {}
                for k in range(nko):
                    # gated loads stay on sync: on scalar/gpsimd they would
                    # head-of-line-block exp/broadcast until the AG lands
                    t = sb.tile([P, CHUNK], BF16, tag=f"ag{k}",
                                name=f"ag{ci}_{k}", bufs=3)
                    nc.sync.dma_start(t[:], bounce_out[k * P:(k + 1) * P, :])
                    tiles[k] = t
                return tiles

            def emit_ag_cols(ci, pair, lo, hi, tiles, spread):
                # partial AllGather: one head pair of chunk `ci`, columns
                # [lo, hi).  The last chunk's pair B splits in two so the
                # final gather carries half the data and fires earlier.
                c0 = ci * CHUNK
                w = hi - lo
                bounce_in = dram.tile([P, w], BF16, tag=f"binh{pair}",
                                      name=f"binh{ci}_{pair}_{lo}", bufs=2)
                bounce_out = dram.tile([grp * P, w], BF16,
                                       tag=f"bouth{pair}",
                                       name=f"bouth{ci}_{pair}_{lo}", bufs=2)
                # pair-B bounce rides gpsimd right before its trigger (sync
                # may still hold pair-A's gated tile loads)
                beng = nc.gpsimd if pair == 1 else nc.sync
                beng.dma_start(bounce_in[:], ot_sb[pair][:, c0 + lo:c0 + hi])
                nc.gpsimd.collective_compute(
                    "AllGather", mybir.AluOpType.bypass,
                    replica_groups=groups,
                    ins=[bounce_in.opt()], outs=[bounce_out.opt()])
                for r in range(grp):
                    k = 2 * r + pair
                    if k not in tiles:
                        tiles[k] = sb.tile([P, CHUNK], BF16, tag=f"ag{k}",
                                           name=f"ag{ci}_{k}", bufs=3)
                    # only the very last gather's loads spread over all
                    # queues (nothing left to block); earlier gated loads
                    # must stay on sync
                    eng = dma_engines[r % 3] if spread else nc.sync
                    eng.dma_start(tiles[k][:, lo:hi],
                                  bounce_out[r * P:(r + 1) * P, :])

            # ---- upfront projections: chunk-0/1 Q, chunk-0 K, chunk-0 V
            # (Q/K/V for later chunks are deferred into the work queue).
            for pair in range(2):
                emit_qt(pair, 0)
            for pair in range(2):
                emit_kt(pair, 0)
            for jt in range(jpc):
                emit_v(jt)
            if nch > 1:
                for pair in range(2):
                    emit_qt(pair, 1)
            for ch in range(2, nch):
                for pair in range(2):
                    work_early.append(
                        lambda pair=pair, ch=ch: emit_qt(pair, ch))

            # ---- attention chunks ----
            last_parts = {}
            for ci in range(nch):
                jt_end = jpc * (ci + 1)
                c0 = ci * CHUNK
                last = ci == nch - 1

                if ci + 1 < nch:
                    for pair in range(2):
                        work_early.append(
                            lambda pair=pair, ch=ci + 1: emit_kt(pair, ch))
                    for jt in range(jpc * (ci + 1), jpc * (ci + 2)):
                        work_early.append(lambda jt=jt: emit_v(jt))

                binf = None if last else dram.tile(
                    [2 * P, CHUNK], BF16, tag="binf", name=f"binf{ci}",
                    bufs=2)

                def normalize(hpass, ot_ps, lo, hi):
                    # rcp of the sums row (both heads fused, columns
                    # [lo,hi)), gpsimd partition-broadcast, then one
                    # psum-read mul per head writes normalized O^T to SBUF.
                    # Everything stays off ACT: exp thruput is the pipeline
                    # rate limiter.
                    w = hi - lo
                    sr2 = sb.tile([1, 2 * w], F32, tag="sr",
                                  name=f"sr{ci}_{hpass}_{lo}", bufs=2)
                    rcp2 = sb.tile([1, 2 * w], F32, tag="rcp",
                                   name=f"rcp{ci}_{hpass}_{lo}", bufs=2)
                    nc.vector.tensor_copy(sr2[:, 0:w],
                                          ot_ps[0][DH:DHA, lo:hi])
                    nc.vector.tensor_copy(sr2[:, w:2 * w],
                                          ot_ps[1][DH:DHA, lo:hi])
                    nc.vector.reciprocal_approx_fast(rcp2[:], sr2[:])
                    bc2 = sb.tile([DH, 2 * w], F32, tag="bc",
                                  name=f"bc{ci}_{hpass}_{lo}", bufs=2)
                    nc.gpsimd.partition_broadcast(bc2[:], rcp2[:],
                                                  channels=DH)
                    for h2 in range(2):
                        nc.vector.tensor_mul(
                            ot_sb[hpass][h2 * DH:(h2 + 1) * DH,
                                         c0 + lo:c0 + hi],
                            ot_ps[h2][0:DH, lo:hi],
                            bc2[:, h2 * w:(h2 + 1) * w])

                HC = CHUNK // 2
                for hpass in range(2):
                    # heads 2*hpass, 2*hpass+1  (== head pair `hpass`)
                    ot_ps = [ps.tile([DHA, CHUNK], F32, tag=f"ot{h2}",
                                     name=f"ot{ci}_{hpass}_{h2}", bufs=1)
                             for h2 in range(2)]

                    def emit_st(jt):
                        # S^T tile = K_h @ Q_h^T (row-tiled, K=64) + exp
                        # for both heads + the diagonal band mask
                        rel = max(0, (jt - jpc * ci)) * P
                        s2 = ps.tile([P, 2 * CHUNK], F32, tag="s2",
                                     name=f"s{ci}_{hpass}_{jt}", bufs=2)
                        es = sb.tile([P, 2 * CHUNK], BF16, tag="es",
                                     name=f"es{ci}_{hpass}_{jt}", bufs=3)
                        for h2 in range(2):
                            nc.tensor.matmul(
                                s2[:, h2 * CHUNK + rel:(h2 + 1) * CHUNK],
                                lhsT=kt_sb[hpass][h2 * DH:(h2 + 1) * DH,
                                                  jt * P:(jt + 1) * P],
                                rhs=qt_sb[hpass][h2 * DH:(h2 + 1) * DH,
                                                 c0 + rel:c0 + CHUNK],
                                start=True, stop=True,
                                tile_position=(h2 * DH, 0))
                        nc.scalar.activation(
                            es.rearrange("p (t c) -> p t c", t=2)[:, :, rel:],
                            s2.rearrange("p (t c) -> p t c", t=2)[:, :, rel:],
                            ActFn.Exp)
                        if jt >= jpc * ci:
                            nc.vector.tensor_mul(
                                es.rearrange("p (t c) -> p t c",
                                             t=2)[:, :, rel:rel + P],
                                es.rearrange("p (t c) -> p t c",
                                             t=2)[:, :, rel:rel + P],
                                mask_sb.rearrange(
                                    "p (o c) -> p o c",
                                    o=1).broadcast_to((P, 2, P)))
                        return es

                    # software pipeline: S^T(jt+1) and fill work are queued
                    # between S^T(jt) and the exp-gated PV(jt) so the
                    # in-order PE queue never blocks on ACT latency
                    es_cur = emit_st(0)
                    for jt in range(jt_end):
                        rel = max(0, (jt - jpc * ci)) * P
                        es_nxt = emit_st(jt + 1) if jt + 1 < jt_end else None
                        # reservoir: drain gated projections only when their
                        # AllGather is surely complete; keep items back for
                        # the final AllGather's flight window
                        pop_work(late_ok=(hpass == 1 or last),
                                 late_floor=(3 if hpass == 1 else
                                             (3 if last else 4)))
                        for h2 in range(2):
                            h = 2 * hpass + h2
                            # O^T(+sums) accumulation: V_aug^T @ expS^T
                            nc.tensor.matmul(
                                ot_ps[h2][:, rel:CHUNK],
                                lhsT=v_sb[jt][:, h * DHA:(h + 1) * DHA],
                                rhs=es_cur[:, h2 * CHUNK + rel:
                                           (h2 + 1) * CHUNK],
                                start=(jt == 0), stop=(jt == jt_end - 1))
                        es_cur = es_nxt

                    normalize(hpass, ot_ps, 0, CHUNK)
                    if last:
                        emit_ag_cols(ci, hpass, 0, CHUNK, last_parts,
                                     spread=(hpass == 1))
                    else:
                        nc.sync.dma_start(
                            binf[hpass * P:(hpass + 1) * P, :],
                            ot_sb[hpass][:, c0:c0 + CHUNK])

                if last:
                    agt = dict(last_parts)
                    last_parts = {}
                else:
                    agt = emit_ag_full(ci, binf)
                evens = [(k, agt[k], 0) for k in range(0, nko, 2)]
                odds = [(k, agt[k], 0) for k in range(1, nko, 2)]
                nm = HS // P
                if not last:
                    for m in range(nm):
                        work_late.append(
                            lambda ci=ci, m=m, s=evens + odds:
                            emit_proj(ci, m, s))
                else:
                    # three-stage projection for the last chunk: evens
                    # (pair-A AG, long landed), odd-left (first half-gather)
                    # and odd-right (final half-gather) — each stage's items
                    # fill the next gather's flight window; psum held across
                    # stages (no other psum users remain at the tail)
                    op_tiles = {}

                    def proj_ev(ci, m):
                        op_tiles[m] = emit_proj(ci, m, evens, evac=False)

                    def proj_od(ci, m):
                        emit_proj(ci, m, odds, op_ps=op_tiles.pop(m))

                    for fn in (proj_ev, proj_od):
                        for m in range(nm):
                            work_late.append(
                                lambda ci=ci, m=m, fn=fn: fn(ci, m))

            # tail: reservoir drains now — the reserved projections and the
            # pair-A halves fill the final AllGather's flight window before
            # the gated pair-B halves run
            while work_early or work_late:
                pop_work(late_ok=True, late_floor=0)

    if compile:
        nc.compile()
    return nc


def _pack(a, p=P):
    # [n*p, c] -> [p, n*c] partition-major repack
    n = a.shape[0] // p
    return np.ascontiguousarray(
        a.reshape(n, p, a.shape[1]).transpose(1, 0, 2).reshape(p, -1))


def make_in_maps(x, Wq, Wk, Wv, Wo, n_cores=N_CORES):
    import ml_dtypes
    bf16 = ml_dtypes.bfloat16
    scale = np.float32(DH ** -0.5)
    # band mask for the diagonal j-tile of S^T [j,i]: keep j <= i
    mask_b = np.triu(np.ones((P, P), np.float32)).astype(bf16)
    in_maps = []
    for c in range(n_cores):
        b, r = divmod(c, 4)
        hs = r * HS
        in_maps.append({
            "xT": _pack(np.ascontiguousarray(x[b].T)).astype(bf16),
            "wq": _pack(Wq[:, hs:hs + HS] * scale).astype(bf16),
            "wk": _pack(Wk[:, hs:hs + HS]).astype(bf16),
            "wv": _pack(Wv[:, hs:hs + HS]).astype(bf16),
            "wo": _pack(Wo[:, hs:hs + HS]).astype(bf16),
            "mask_c": mask_b,
        })
    return in_maps


def assemble_out(results, B, seq, n_cores=N_CORES):
    out = np.empty((B, seq, INNER), np.float32)
    for c in range(n_cores):
        b, r = divmod(c, 4)
        out[b][:, r * HS:(r + 1) * HS] = results[c]["outT"].T.astype(np.float32)
    return out


_NC_CACHE = {}


def kernel(x, Wq, Wk, Wv, Wo):
    from concourse import bass_utils
    x = np.asarray(x, np.float32)
    B, seq, dim = x.shape
    key = (seq, dim)
    if key not in _NC_CACHE:
        _NC_CACHE[key] = build_nc(seq=seq, dim=dim)
    nc = _NC_CACHE[key]
    in_maps = make_in_maps(x, np.asarray(Wq, np.float32),
                           np.asarray(Wk, np.float32),
                           np.asarray(Wv, np.float32),
                           np.asarray(Wo, np.float32))
    res = bass_utils.run_bass_kernel_spmd(
        nc, in_maps, core_ids=list(range(N_CORES)))
    return assemble_out(res.results, B, seq)


# revision 40
# speedup vs baseline: 1.0208x; 1.0208x over previous
"""Distributed causal multi-head attention for one TRN2 chip (8 NeuronCores).

Sharding: batch (2) x head-groups (4 heads/core) -> 8 cores.
Core c handles batch c//4, heads [ (c%4)*4 , (c%4)*4+4 ).
Per core: QKV projections for its 4 heads, flash-style causal attention
with scores kept transposed (S^T = K @ Q^T) so the PV product needs no
transposes; V is augmented with a ones column so the softmax denominators
fall out of the same matmul (row 64 of each head's O^T psum).  Then an
AllGather of the attention output (pre-Wo, 4-core group = one batch) and
a column-sliced output projection.  Host assembles the 8 column/batch
shards.  Compute dtype bf16 (PSUM accumulation fp32), softmax in fp32.

Scheduling notes:
- All host inputs are repacked partition-major so every input tensor
  loads with a handful of large contiguous DMAs.
- The attention pipeline is ACT(exp)-rate-limited; KT/V/Q projections
  for later chunks and AllGather-gated output projections are queued
  work items drained into the PE's idle slots.
- AllGather-gated DMA loads ride the sync queue only: at the head of
  the scalar/gpsimd queue they would block exp/broadcast behind them.
- Projections are drained lazily (reservoir) so PE work remains to fill
  AllGather flight windows; the flight of the final half-chunk gather
  is covered by reserved projections plus the pair-A halves of the last
  chunk's projections (held psum accumulation), keeping the HAM clock
  gate warm through the tail.
"""

import sys
from collections import deque

import numpy as np

sys.path.insert(0, "/opt/trn_rl_repo")

import concourse.bass as bass  # noqa: E402
import concourse.bacc as bacc  # noqa: E402
import concourse.tile as tile  # noqa: E402
import concourse.mybir as mybir  # noqa: E402

F32 = mybir.dt.float32
BF16 = mybir.dt.bfloat16
ActFn = mybir.ActivationFunctionType

P = 128          # partition dim
CHUNK = 512      # i-chunk (matmul moving free dim, one psum bank of fp32)
DH = 64          # head dim
HPC = 4          # heads per core
HS = HPC * DH    # 256 per-core inner slice
DHA = DH + 1     # augmented head dim (ones column for softmax sums)
INNER = 1024     # total inner dim (16 heads x 64)
N_CORES = 8
GROUPS = [[0, 1, 2, 3], [4, 5, 6, 7]]


def build_nc(seq=2048, dim=1024, n_cores=N_CORES, groups=GROUPS, compile=True):
    """Build the SPMD Bass graph (identical on all cores)."""
    nch = seq // CHUNK          # i-chunks
    jpc = CHUNK // P            # j-tiles per chunk (4)
    njt = seq // P              # j-tiles
    nk = dim // P               # feature k-tiles
    nko = INNER // P            # inner k-tiles for the output projection
    grp = len(groups[0])        # replica group size (4)

    nc = bacc.Bacc("TRN2", target_bir_lowering=False, debug=False,
                   enable_asserts=False, num_devices=n_cores)

    # all inputs are host-repacked partition-major: [P, ntiles*cols]
    xT = nc.dram_tensor("xT", [P, nk * seq], BF16, kind="ExternalInput").ap()
    wq = nc.dram_tensor("wq", [P, nk * HS], BF16, kind="ExternalInput").ap()
    wk = nc.dram_tensor("wk", [P, nk * HS], BF16, kind="ExternalInput").ap()
    wv = nc.dram_tensor("wv", [P, nk * HS], BF16, kind="ExternalInput").ap()
    wo = nc.dram_tensor("wo", [P, nko * HS], BF16, kind="ExternalInput").ap()
    mask_c = nc.dram_tensor("mask_c", [P, P], BF16, kind="ExternalInput").ap()
    outT = nc.dram_tensor("outT", [HS, seq], BF16, kind="ExternalOutput").ap()

    with tile.TileContext(nc) as tc:
        with tc.tile_pool(name="sb", bufs=1) as sb, \
             tc.tile_pool(name="ps", bufs=1, space="PSUM") as ps, \
             tc.tile_pool(name="dram", bufs=1, space="DRAM") as dram:

            dma_engines = [nc.sync, nc.scalar, nc.gpsimd]

            # ---- input loads: few, large, contiguous DMAs; xt halves
            # k-grouped across the three DMA queues so chunk-0 columns of
            # every k-tile land within the first few microseconds ----
            xts = sb.tile([P, nk * seq], BF16, tag="xts", name="xts")
            xt = [xts[:, k * seq:(k + 1) * seq] for k in range(nk)]
            wqs = sb.tile([P, nk * HS], BF16, tag="wqs", name="wqs")
            wq_sb = [wqs[:, k * HS:(k + 1) * HS] for k in range(nk)]
            wks = sb.tile([P, nk * HS], BF16, tag="wks", name="wks")
            wk_sb = [wks[:, k * HS:(k + 1) * HS] for k in range(nk)]
            wvs = sb.tile([P, nk * HS], BF16, tag="wvs", name="wvs")
            wv_sb = [wvs[:, k * HS:(k + 1) * HS] for k in range(nk)]
            wos = sb.tile([P, nko * HS], BF16, tag="wos", name="wos")
            wo_sb = [wos[:, k * HS:(k + 1) * HS] for k in range(nko)]
            mask_sb = sb.tile([P, P], BF16, tag="mask", name="mask")

            # half-width per-k loads: 2KB contiguous runs (full DMA rate),
            # per-tile semaphores so the QKV k-loops stream as tiles land
            half = seq // 2
            wh = nk * HS // 2
            nc.gpsimd.dma_start(mask_sb[:], mask_c[:])
            nc.sync.dma_start(wqs[:, 0:wh], wq[:, 0:wh])
            nc.scalar.dma_start(wqs[:, wh:], wq[:, wh:])
            for k in range(nk):
                dma_engines[k % 3].dma_start(
                    xt[k][:, 0:half], xT[:, k * seq:k * seq + half])
                if k == min(2, nk - 1):
                    nc.sync.dma_start(wks[:, 0:wh], wk[:, 0:wh])
                    nc.scalar.dma_start(wks[:, wh:], wk[:, wh:])
                    nc.gpsimd.dma_start(wvs[:], wv[:])
            for k in range(nk):
                dma_engines[k % 3].dma_start(
                    xt[k][:, half:seq], xT[:, k * seq + half:(k + 1) * seq])
            nc.scalar.dma_start(wos[:], wo[:])

            # warm up the collectives firmware (tiny: the entry barrier's
            # length varies run to run, so a real-sized warmup can land in
            # the critical CC window).  Emitted after the input loads: its
            # trigger waits on the warm_in DMA and would otherwise block
            # the gpsimd queue's share of the input loads.
            warm_in = dram.tile([P, 4], BF16, tag="warm_i", name="warm_i")
            warm_out = dram.tile([grp * P, 4], BF16,
                                 tag="warm_o", name="warm_o")
            nc.gpsimd.dma_start(warm_in[:], xT[0:P, 0:4])
            nc.gpsimd.collective_compute(
                "AllGather", mybir.AluOpType.bypass, replica_groups=groups,
                ins=[warm_in.opt()], outs=[warm_out.opt()])

            # persistent QKV results
            qt_sb = [sb.tile([P, seq], BF16, tag=f"qt{p}", name=f"qt{p}")
                     for p in range(2)]
            kt_sb = [sb.tile([P, seq], BF16, tag=f"kt{p}", name=f"kt{p}")
                     for p in range(2)]
            v_sb = [sb.tile([P, HPC * DHA], BF16, tag=f"v{j}", name=f"v{j}")
                    for j in range(njt)]
            ot_sb = [sb.tile([P, seq], BF16, tag=f"ot{p}", name=f"ot{p}")
                     for p in range(2)]

            # ---- interleavable work items (each emits one psum group) ----
            def emit_kt(pair, ch):
                pt = ps.tile([P, CHUNK], F32, tag="misc",
                             name=f"ktps{pair}_{ch}", bufs=2)
                for k in range(nk):
                    nc.tensor.matmul(
                        pt[:], lhsT=wk_sb[k][:, pair * P:(pair + 1) * P],
                        rhs=xt[k][:, ch * CHUNK:(ch + 1) * CHUNK],
                        start=(k == 0), stop=(k == nk - 1))
                nc.vector.tensor_copy(
                    kt_sb[pair][:, ch * CHUNK:(ch + 1) * CHUNK], pt[:])

            def emit_v(jt):
                pt = ps.tile([P, HS], F32, tag="misc",
                             name=f"vps{jt}", bufs=2)
                for k in range(nk):
                    nc.tensor.matmul(
                        pt[:], lhsT=xt[k][:, jt * P:(jt + 1) * P],
                        rhs=wv_sb[k][:],
                        start=(k == 0), stop=(k == nk - 1))
                nc.vector.tensor_copy(
                    v_sb[jt].rearrange("p (h d) -> p h d", h=HPC)[:, :, 0:DH],
                    pt.rearrange("p (h d) -> p h d", h=HPC))
                nc.vector.memset(
                    v_sb[jt].rearrange("p (h d) -> p h d", h=HPC)[:, :, DH:DHA],
                    1.0)

            def emit_qt(pair, ch):
                pt = ps.tile([P, CHUNK], F32, tag="misc",
                             name=f"qps{pair}_{ch}", bufs=2)
                for k in range(nk):
                    nc.tensor.matmul(
                        pt[:],
                        lhsT=wq_sb[k][:, pair * P:(pair + 1) * P],
                        rhs=xt[k][:, ch * CHUNK:(ch + 1) * CHUNK],
                        start=(k == 0), stop=(k == nk - 1))
                nc.vector.tensor_copy(
                    qt_sb[pair][:, ch * CHUNK:(ch + 1) * CHUNK], pt[:])

            def emit_proj(ci, m, slices, op_ps=None, evac=True,
                          lo=0, hi=CHUNK, stop=None):
                # transposed output block: outT[m*128:(m+1)*128, chunk ci]
                # = Wo[:, m-slice].T @ attT[:, chunk] over the k-tiles in
                # `slices`, columns [lo,hi) (partial passes keep op_ps
                # alive; has_written bits track sub-regions)
                c0 = ci * CHUNK
                first = op_ps is None
                if first:
                    op_ps = ps.tile([P, CHUNK], F32, tag="misc",
                                    name=f"op{ci}_{m}", bufs=2)
                if stop is None:
                    stop = evac
                for n, (k, ag_t, coff) in enumerate(slices):
                    nc.tensor.matmul(
                        op_ps[:, lo:hi],
                        lhsT=wo_sb[k][:, m * P:(m + 1) * P],
                        rhs=ag_t[:, coff + lo:coff + hi],
                        start=(first and n == 0),
                        stop=(stop and n == len(slices) - 1))
                if not evac:
                    return op_ps
                o_sb = sb.tile([P, CHUNK], BF16, tag="osb",
                               name=f"o{ci}_{m}", bufs=4)
                nc.vector.tensor_copy(o_sb[:], op_ps[:])
                nc.sync.dma_start(outT[m * P:(m + 1) * P, c0:c0 + CHUNK],
                                  o_sb[:])
                return None

            work_early = deque()   # KT/V/Q for future chunks (not gated)
            work_late = deque()    # output projections (gated on AllGather)

            def pop_work(late_ok, late_floor):
                if work_early:
                    work_early.popleft()()
                elif late_ok and len(work_late) > late_floor:
                    work_late.popleft()()

            def emit_ag_full(ci, bounce_in):
                # one AllGather for both head pairs of chunk `ci` (256KB —
                # amortizes the ncfw floor; rank-major rows land so that
                # gathered row-block k*128 is exactly attT k-tile k)
                bounce_out = dram.tile([grp * 2 * P, CHUNK], BF16,
                                       tag="boutf", name=f"boutf{ci}", bufs=2)
                nc.gpsimd.collective_compute(
                    "AllGather", mybir.AluOpType.bypass,
                    replica_groups=groups,
                    ins=[bounce_in.opt()], outs=[bounce_out.opt()])
                tiles = {}
                for k in range(nko):
                    # gated loads stay on sync: on scalar/gpsimd they would
                    # head-of-line-block exp/broadcast until the AG lands
                    t = sb.tile([P, CHUNK], BF16, tag=f"ag{k}",
                                name=f"ag{ci}_{k}", bufs=3)
                    nc.sync.dma_start(t[:], bounce_out[k * P:(k + 1) * P, :])
                    tiles[k] = t
                return tiles

            def emit_ag_cols(ci, pair, lo, hi, tiles, spread):
                # partial AllGather: one head pair of chunk `ci`, columns
                # [lo, hi).  The last chunk's pair B splits in two so the
                # final gather carries half the data and fires earlier.
                c0 = ci * CHUNK
                w = hi - lo
                bounce_in = dram.tile([P, w], BF16, tag=f"binh{pair}",
                                      name=f"binh{ci}_{pair}_{lo}", bufs=2)
                bounce_out = dram.tile([grp * P, w], BF16,
                                       tag=f"bouth{pair}",
                                       name=f"bouth{ci}_{pair}_{lo}", bufs=2)
                # pair-B bounce rides gpsimd right before its trigger (sync
                # may still hold pair-A's gated tile loads)
                beng = nc.gpsimd if pair == 1 else nc.sync
                beng.dma_start(bounce_in[:], ot_sb[pair][:, c0 + lo:c0 + hi])
                nc.gpsimd.collective_compute(
                    "AllGather", mybir.AluOpType.bypass,
                    replica_groups=groups,
                    ins=[bounce_in.opt()], outs=[bounce_out.opt()])
                for r in range(grp):
                    k = 2 * r + pair
                    if k not in tiles:
                        tiles[k] = sb.tile([P, CHUNK], BF16, tag=f"ag{k}",
                                           name=f"ag{ci}_{k}", bufs=3)
                    # only the very last gather's loads spread over all
                    # queues (nothing left to block); earlier gated loads
                    # must stay on sync
                    eng = dma_engines[r % 3] if spread else nc.sync
                    eng.dma_start(tiles[k][:, lo:hi],
                                  bounce_out[r * P:(r + 1) * P, :])

            # ---- upfront projections: chunk-0/1 Q, chunk-0 K, chunk-0 V
            # (Q/K/V for later chunks are deferred into the work queue).
            for pair in range(2):
                emit_qt(pair, 0)
            for pair in range(2):
                emit_kt(pair, 0)
            for jt in range(jpc):
                emit_v(jt)
            if nch > 1:
                for pair in range(2):
                    emit_qt(pair, 1)
            for ch in range(2, nch):
                for pair in range(2):
                    work_early.append(
                        lambda pair=pair, ch=ch: emit_qt(pair, ch))

            # ---- attention chunks ----
            last_parts = {}
            for ci in range(nch):
                jt_end = jpc * (ci + 1)
                c0 = ci * CHUNK
                last = ci == nch - 1

                if ci + 1 < nch:
                    for pair in range(2):
                        work_early.append(
                            lambda pair=pair, ch=ci + 1: emit_kt(pair, ch))
                    for jt in range(jpc * (ci + 1), jpc * (ci + 2)):
                        work_early.append(lambda jt=jt: emit_v(jt))

                binf = None if last else dram.tile(
                    [2 * P, CHUNK], BF16, tag="binf", name=f"binf{ci}",
                    bufs=2)

                def normalize(hpass, ot_ps, lo, hi):
                    # rcp of the sums row (both heads fused, columns
                    # [lo,hi)), gpsimd partition-broadcast, then one
                    # psum-read mul per head writes normalized O^T to SBUF.
                    # Everything stays off ACT: exp thruput is the pipeline
                    # rate limiter.
                    w = hi - lo
                    sr2 = sb.tile([1, 2 * w], F32, tag="sr",
                                  name=f"sr{ci}_{hpass}_{lo}", bufs=2)
                    rcp2 = sb.tile([1, 2 * w], F32, tag="rcp",
                                   name=f"rcp{ci}_{hpass}_{lo}", bufs=2)
                    nc.vector.tensor_copy(sr2[:, 0:w],
                                          ot_ps[0][DH:DHA, lo:hi])
                    nc.vector.tensor_copy(sr2[:, w:2 * w],
                                          ot_ps[1][DH:DHA, lo:hi])
                    nc.vector.reciprocal_approx_fast(rcp2[:], sr2[:])
                    bc2 = sb.tile([DH, 2 * w], F32, tag="bc",
                                  name=f"bc{ci}_{hpass}_{lo}", bufs=2)
                    nc.gpsimd.partition_broadcast(bc2[:], rcp2[:],
                                                  channels=DH)
                    for h2 in range(2):
                        nc.vector.tensor_mul(
                            ot_sb[hpass][h2 * DH:(h2 + 1) * DH,
                                         c0 + lo:c0 + hi],
                            ot_ps[h2][0:DH, lo:hi],
                            bc2[:, h2 * w:(h2 + 1) * w])

                HC = CHUNK // 2
                for hpass in range(2):
                    # heads 2*hpass, 2*hpass+1  (== head pair `hpass`)
                    ot_ps = [ps.tile([DHA, CHUNK], F32, tag=f"ot{h2}",
                                     name=f"ot{ci}_{hpass}_{h2}", bufs=1)
                             for h2 in range(2)]

                    def emit_st(jt):
                        # S^T tile = K_h @ Q_h^T (row-tiled, K=64) + exp
                        # for both heads + the diagonal band mask
                        rel = max(0, (jt - jpc * ci)) * P
                        s2 = ps.tile([P, 2 * CHUNK], F32, tag="s2",
                                     name=f"s{ci}_{hpass}_{jt}", bufs=2)
                        es = sb.tile([P, 2 * CHUNK], BF16, tag="es",
                                     name=f"es{ci}_{hpass}_{jt}", bufs=3)
                        for h2 in range(2):
                            nc.tensor.matmul(
                                s2[:, h2 * CHUNK + rel:(h2 + 1) * CHUNK],
                                lhsT=kt_sb[hpass][h2 * DH:(h2 + 1) * DH,
                                                  jt * P:(jt + 1) * P],
                                rhs=qt_sb[hpass][h2 * DH:(h2 + 1) * DH,
                                                 c0 + rel:c0 + CHUNK],
                                start=True, stop=True,
                                tile_position=(h2 * DH, 0))
                        nc.scalar.activation(
                            es.rearrange("p (t c) -> p t c", t=2)[:, :, rel:],
                            s2.rearrange("p (t c) -> p t c", t=2)[:, :, rel:],
                            ActFn.Exp)
                        if jt >= jpc * ci:
                            nc.vector.tensor_mul(
                                es.rearrange("p (t c) -> p t c",
                                             t=2)[:, :, rel:rel + P],
                                es.rearrange("p (t c) -> p t c",
                                             t=2)[:, :, rel:rel + P],
                                mask_sb.rearrange(
                                    "p (o c) -> p o c",
                                    o=1).broadcast_to((P, 2, P)))
                        return es

                    # software pipeline: S^T(jt+1) and fill work are queued
                    # between S^T(jt) and the exp-gated PV(jt) so the
                    # in-order PE queue never blocks on ACT latency
                    es_cur = emit_st(0)
                    for jt in range(jt_end):
                        rel = max(0, (jt - jpc * ci)) * P
                        es_nxt = emit_st(jt + 1) if jt + 1 < jt_end else None
                        # reservoir: drain gated projections only when their
                        # AllGather is surely complete; keep items back for
                        # the final AllGather's flight window
                        pop_work(late_ok=(hpass == 1 or last),
                                 late_floor=(3 if hpass == 1 else
                                             (3 if last else 4)))
                        for h2 in range(2):
                            h = 2 * hpass + h2
                            # O^T(+sums) accumulation: V_aug^T @ expS^T
                            nc.tensor.matmul(
                                ot_ps[h2][:, rel:CHUNK],
                                lhsT=v_sb[jt][:, h * DHA:(h + 1) * DHA],
                                rhs=es_cur[:, h2 * CHUNK + rel:
                                           (h2 + 1) * CHUNK],
                                start=(jt == 0), stop=(jt == jt_end - 1))
                        es_cur = es_nxt

                    normalize(hpass, ot_ps, 0, CHUNK)
                    if last:
                        emit_ag_cols(ci, hpass, 0, CHUNK, last_parts,
                                     spread=(hpass == 1))
                    else:
                        nc.sync.dma_start(
                            binf[hpass * P:(hpass + 1) * P, :],
                            ot_sb[hpass][:, c0:c0 + CHUNK])

                if last:
                    agt = dict(last_parts)
                    last_parts = {}
                else:
                    agt = emit_ag_full(ci, binf)
                evens = [(k, agt[k], 0) for k in range(0, nko, 2)]
                odds = [(k, agt[k], 0) for k in range(1, nko, 2)]
                nm = HS // P
                # schedule-time floors: the static scheduler backfills PE
                # idle slots with any ready-by-its-model work — and its
                # AllGather estimate is optimistic.  A slug AG (25µs+) then
                # head-of-line-blocks the whole PE queue on a gated
                # projection.  Pin gated work past the worst-case landing.
                ag_eta = {0: 0.100, 1: 0.140, 2: 0.178}
                if not last:
                    for m in range(nm):
                        def run_proj(ci=ci, m=m, s=evens + odds):
                            with tc.tile_wait_until(ag_eta[ci]):
                                emit_proj(ci, m, s)
                        work_late.append(run_proj)
                else:
                    # three-stage projection for the last chunk: evens
                    # (pair-A AG, long landed), odd-left (first half-gather)
                    # and odd-right (final half-gather) — each stage's items
                    # fill the next gather's flight window; psum held across
                    # stages (no other psum users remain at the tail)
                    op_tiles = {}

                    def proj_ev(ci, m):
                        with tc.tile_wait_until(0.190):
                            op_tiles[m] = emit_proj(ci, m, evens, evac=False)

                    def proj_od(ci, m):
                        with tc.tile_wait_until(0.206):
                            emit_proj(ci, m, odds, op_ps=op_tiles.pop(m))

                    for fn in (proj_ev, proj_od):
                        for m in range(nm):
                            work_late.append(
                                lambda ci=ci, m=m, fn=fn: fn(ci, m))

            # tail: reservoir drains now — the reserved projections and the
            # pair-A halves fill the final AllGather's flight window before
            # the gated pair-B halves run
            while work_early or work_late:
                pop_work(late_ok=True, late_floor=0)

    if compile:
        nc.compile()
    return nc


def _pack(a, p=P):
    # [n*p, c] -> [p, n*c] partition-major repack
    n = a.shape[0] // p
    return np.ascontiguousarray(
        a.reshape(n, p, a.shape[1]).transpose(1, 0, 2).reshape(p, -1))


def make_in_maps(x, Wq, Wk, Wv, Wo, n_cores=N_CORES):
    import ml_dtypes
    bf16 = ml_dtypes.bfloat16
    scale = np.float32(DH ** -0.5)
    # band mask for the diagonal j-tile of S^T [j,i]: keep j <= i
    mask_b = np.triu(np.ones((P, P), np.float32)).astype(bf16)
    in_maps = []
    for c in range(n_cores):
        b, r = divmod(c, 4)
        hs = r * HS
        in_maps.append({
            "xT": _pack(np.ascontiguousarray(x[b].T)).astype(bf16),
            "wq": _pack(Wq[:, hs:hs + HS] * scale).astype(bf16),
            "wk": _pack(Wk[:, hs:hs + HS]).astype(bf16),
            "wv": _pack(Wv[:, hs:hs + HS]).astype(bf16),
            "wo": _pack(Wo[:, hs:hs + HS]).astype(bf16),
            "mask_c": mask_b,
        })
    return in_maps


def assemble_out(results, B, seq, n_cores=N_CORES):
    out = np.empty((B, seq, INNER), np.float32)
    for c in range(n_cores):
        b, r = divmod(c, 4)
        out[b][:, r * HS:(r + 1) * HS] = results[c]["outT"].T.astype(np.float32)
    return out


_NC_CACHE = {}


def kernel(x, Wq, Wk, Wv, Wo):
    from concourse import bass_utils
    x = np.asarray(x, np.float32)
    B, seq, dim = x.shape
    key = (seq, dim)
    if key not in _NC_CACHE:
        _NC_CACHE[key] = build_nc(seq=seq, dim=dim)
    nc = _NC_CACHE[key]
    in_maps = make_in_maps(x, np.asarray(Wq, np.float32),
                           np.asarray(Wk, np.float32),
                           np.asarray(Wv, np.float32),
                           np.asarray(Wo, np.float32))
    res = bass_utils.run_bass_kernel_spmd(
        nc, in_maps, core_ids=list(range(N_CORES)))
    return assemble_out(res.results, B, seq)


# revision 42
# speedup vs baseline: 1.1575x; 1.1338x over previous
"""Distributed causal multi-head attention for one TRN2 chip (8 NeuronCores).

Sharding: batch (2) x head-groups (4 heads/core) -> 8 cores.
Core c handles batch c//4, heads [ (c%4)*4 , (c%4)*4+4 ).
Per core: QKV projections for its 4 heads, flash-style causal attention
with scores kept transposed (S^T = K @ Q^T) so the PV product needs no
transposes; V is augmented with a ones column so the softmax denominators
fall out of the same matmul (row 64 of each head's O^T psum).  Then an
AllGather of the attention output (pre-Wo, 4-core group = one batch) and
a column-sliced output projection.  Host assembles the 8 column/batch
shards.  Compute dtype bf16 (PSUM accumulation fp32), softmax in fp32.

Scheduling notes:
- All host inputs are repacked partition-major so every input tensor
  loads with a handful of large contiguous DMAs.
- The attention pipeline is ACT(exp)-rate-limited; KT/V/Q projections
  for later chunks and AllGather-gated output projections are queued
  work items drained into the PE's idle slots.
- AllGather-gated DMA loads ride the sync queue only: at the head of
  the scalar/gpsimd queue they would block exp/broadcast behind them.
- Projections are drained lazily (reservoir) so PE work remains to fill
  AllGather flight windows; the flight of the final half-chunk gather
  is covered by reserved projections plus the pair-A halves of the last
  chunk's projections (held psum accumulation), keeping the HAM clock
  gate warm through the tail.
"""

import sys
from collections import deque

import numpy as np

sys.path.insert(0, "/opt/trn_rl_repo")

import concourse.bass as bass  # noqa: E402
import concourse.bacc as bacc  # noqa: E402
import concourse.tile as tile  # noqa: E402
import concourse.mybir as mybir  # noqa: E402

F32 = mybir.dt.float32
BF16 = mybir.dt.bfloat16
ActFn = mybir.ActivationFunctionType

P = 128          # partition dim
CHUNK = 512      # i-chunk (matmul moving free dim, one psum bank of fp32)
DH = 64          # head dim
HPC = 4          # heads per core
HS = HPC * DH    # 256 per-core inner slice
DHA = DH + 1     # augmented head dim (ones column for softmax sums)
INNER = 1024     # total inner dim (16 heads x 64)
N_CORES = 8
GROUPS = [[0, 1, 2, 3], [4, 5, 6, 7]]


def build_nc(seq=2048, dim=1024, n_cores=N_CORES, groups=GROUPS, compile=True):
    """Build the SPMD Bass graph (identical on all cores)."""
    nch = seq // CHUNK          # i-chunks
    jpc = CHUNK // P            # j-tiles per chunk (4)
    njt = seq // P              # j-tiles
    nk = dim // P               # feature k-tiles
    nko = INNER // P            # inner k-tiles for the output projection
    grp = len(groups[0])        # replica group size (4)

    nc = bacc.Bacc("TRN2", target_bir_lowering=False, debug=False,
                   enable_asserts=False, num_devices=n_cores)

    # all inputs are host-repacked partition-major: [P, ntiles*cols]
    xT = nc.dram_tensor("xT", [P, nk * seq], BF16, kind="ExternalInput").ap()
    wq = nc.dram_tensor("wq", [P, nk * HS], BF16, kind="ExternalInput").ap()
    wk = nc.dram_tensor("wk", [P, nk * HS], BF16, kind="ExternalInput").ap()
    wv = nc.dram_tensor("wv", [P, nk * HS], BF16, kind="ExternalInput").ap()
    wo = nc.dram_tensor("wo", [P, nko * HS], BF16, kind="ExternalInput").ap()
    mask_c = nc.dram_tensor("mask_c", [P, P], BF16, kind="ExternalInput").ap()
    outT = nc.dram_tensor("outT", [HS, seq], BF16, kind="ExternalOutput").ap()

    with tile.TileContext(nc) as tc:
        with tc.tile_pool(name="sb", bufs=1) as sb, \
             tc.tile_pool(name="ps", bufs=1, space="PSUM") as ps, \
             tc.tile_pool(name="dram", bufs=1, space="DRAM") as dram:

            dma_engines = [nc.sync, nc.scalar, nc.gpsimd]

            # ---- input loads: few, large, contiguous DMAs; xt halves
            # k-grouped across the three DMA queues so chunk-0 columns of
            # every k-tile land within the first few microseconds ----
            xts = sb.tile([P, nk * seq], BF16, tag="xts", name="xts")
            xt = [xts[:, k * seq:(k + 1) * seq] for k in range(nk)]
            wqs = sb.tile([P, nk * HS], BF16, tag="wqs", name="wqs")
            wq_sb = [wqs[:, k * HS:(k + 1) * HS] for k in range(nk)]
            wks = sb.tile([P, nk * HS], BF16, tag="wks", name="wks")
            wk_sb = [wks[:, k * HS:(k + 1) * HS] for k in range(nk)]
            wvs = sb.tile([P, nk * HS], BF16, tag="wvs", name="wvs")
            wv_sb = [wvs[:, k * HS:(k + 1) * HS] for k in range(nk)]
            wos = sb.tile([P, nko * HS], BF16, tag="wos", name="wos")
            wo_sb = [wos[:, k * HS:(k + 1) * HS] for k in range(nko)]
            mask_sb = sb.tile([P, P], BF16, tag="mask", name="mask")

            # half-width per-k loads: 2KB contiguous runs (full DMA rate),
            # per-tile semaphores so the QKV k-loops stream as tiles land
            half = seq // 2
            wh = nk * HS // 2
            nc.gpsimd.dma_start(mask_sb[:], mask_c[:])
            nc.sync.dma_start(wqs[:, 0:wh], wq[:, 0:wh])
            nc.scalar.dma_start(wqs[:, wh:], wq[:, wh:])
            for k in range(nk):
                dma_engines[k % 3].dma_start(
                    xt[k][:, 0:half], xT[:, k * seq:k * seq + half])
                if k == min(2, nk - 1):
                    nc.sync.dma_start(wks[:, 0:wh], wk[:, 0:wh])
                    nc.scalar.dma_start(wks[:, wh:], wk[:, wh:])
                    nc.gpsimd.dma_start(wvs[:], wv[:])
            for k in range(nk):
                dma_engines[k % 3].dma_start(
                    xt[k][:, half:seq], xT[:, k * seq + half:(k + 1) * seq])
            nc.scalar.dma_start(wos[:], wo[:])

            # warm up the collectives firmware (tiny: the entry barrier's
            # length varies run to run, so a real-sized warmup can land in
            # the critical CC window).  Emitted after the input loads: its
            # trigger waits on the warm_in DMA and would otherwise block
            # the gpsimd queue's share of the input loads.
            warm_in = dram.tile([P, 4], BF16, tag="warm_i", name="warm_i")
            warm_out = dram.tile([grp * P, 4], BF16,
                                 tag="warm_o", name="warm_o")
            nc.gpsimd.dma_start(warm_in[:], xT[0:P, 0:4])
            nc.gpsimd.collective_compute(
                "AllGather", mybir.AluOpType.bypass, replica_groups=groups,
                ins=[warm_in.opt()], outs=[warm_out.opt()])

            # persistent QKV results
            qt_sb = [sb.tile([P, seq], BF16, tag=f"qt{p}", name=f"qt{p}")
                     for p in range(2)]
            kt_sb = [sb.tile([P, seq], BF16, tag=f"kt{p}", name=f"kt{p}")
                     for p in range(2)]
            v_sb = [sb.tile([P, HPC * DHA], BF16, tag=f"v{j}", name=f"v{j}")
                    for j in range(njt)]
            ot_sb = [sb.tile([P, seq], BF16, tag=f"ot{p}", name=f"ot{p}")
                     for p in range(2)]

            # ---- interleavable work items (each emits one psum group) ----
            def emit_kt(pair, ch):
                pt = ps.tile([P, CHUNK], F32, tag="misc",
                             name=f"ktps{pair}_{ch}", bufs=2)
                for k in range(nk):
                    nc.tensor.matmul(
                        pt[:], lhsT=wk_sb[k][:, pair * P:(pair + 1) * P],
                        rhs=xt[k][:, ch * CHUNK:(ch + 1) * CHUNK],
                        start=(k == 0), stop=(k == nk - 1))
                nc.vector.tensor_copy(
                    kt_sb[pair][:, ch * CHUNK:(ch + 1) * CHUNK], pt[:])

            def emit_v(jt):
                pt = ps.tile([P, HS], F32, tag="misc",
                             name=f"vps{jt}", bufs=2)
                for k in range(nk):
                    nc.tensor.matmul(
                        pt[:], lhsT=xt[k][:, jt * P:(jt + 1) * P],
                        rhs=wv_sb[k][:],
                        start=(k == 0), stop=(k == nk - 1))
                nc.vector.tensor_copy(
                    v_sb[jt].rearrange("p (h d) -> p h d", h=HPC)[:, :, 0:DH],
                    pt.rearrange("p (h d) -> p h d", h=HPC))
                nc.vector.memset(
                    v_sb[jt].rearrange("p (h d) -> p h d", h=HPC)[:, :, DH:DHA],
                    1.0)

            def emit_qt(pair, ch):
                pt = ps.tile([P, CHUNK], F32, tag="misc",
                             name=f"qps{pair}_{ch}", bufs=2)
                for k in range(nk):
                    nc.tensor.matmul(
                        pt[:],
                        lhsT=wq_sb[k][:, pair * P:(pair + 1) * P],
                        rhs=xt[k][:, ch * CHUNK:(ch + 1) * CHUNK],
                        start=(k == 0), stop=(k == nk - 1))
                nc.vector.tensor_copy(
                    qt_sb[pair][:, ch * CHUNK:(ch + 1) * CHUNK], pt[:])

            def emit_proj(ci, m, slices, op_ps=None, evac=True,
                          lo=0, hi=CHUNK, stop=None):
                # transposed output block: outT[m*128:(m+1)*128, chunk ci]
                # = Wo[:, m-slice].T @ attT[:, chunk] over the k-tiles in
                # `slices`, columns [lo,hi) (partial passes keep op_ps
                # alive; has_written bits track sub-regions)
                c0 = ci * CHUNK
                first = op_ps is None
                if first:
                    op_ps = ps.tile([P, CHUNK], F32, tag="misc",
                                    name=f"op{ci}_{m}", bufs=2)
                if stop is None:
                    stop = evac
                for n, (k, ag_t, coff) in enumerate(slices):
                    nc.tensor.matmul(
                        op_ps[:, lo:hi],
                        lhsT=wo_sb[k][:, m * P:(m + 1) * P],
                        rhs=ag_t[:, coff + lo:coff + hi],
                        start=(first and n == 0),
                        stop=(stop and n == len(slices) - 1))
                if not evac:
                    return op_ps
                o_sb = sb.tile([P, CHUNK], BF16, tag="osb",
                               name=f"o{ci}_{m}", bufs=4)
                nc.vector.tensor_copy(o_sb[:], op_ps[:])
                nc.sync.dma_start(outT[m * P:(m + 1) * P, c0:c0 + CHUNK],
                                  o_sb[:])
                return None

            work_early = deque()   # KT/V/Q for future chunks (not gated)
            work_late = deque()    # output projections (gated on AllGather)

            def pop_work(late_ok, late_floor):
                if work_early:
                    work_early.popleft()()
                elif late_ok and len(work_late) > late_floor:
                    work_late.popleft()()

            def emit_ag_full(ci, bounce_in):
                # one AllGather for both head pairs of chunk `ci` (256KB —
                # amortizes the ncfw floor; rank-major rows land so that
                # gathered row-block k*128 is exactly attT k-tile k)
                bounce_out = dram.tile([grp * 2 * P, CHUNK], BF16,
                                       tag="boutf", name=f"boutf{ci}", bufs=2)
                nc.gpsimd.collective_compute(
                    "AllGather", mybir.AluOpType.bypass,
                    replica_groups=groups,
                    ins=[bounce_in.opt()], outs=[bounce_out.opt()])
                tiles = {}
                for k in range(nko):
                    # gated loads stay on sync: on scalar/gpsimd they would
                    # head-of-line-block exp/broadcast until the AG lands
                    t = sb.tile([P, CHUNK], BF16, tag=f"ag{k}",
                                name=f"ag{ci}_{k}", bufs=3)
                    nc.sync.dma_start(t[:], bounce_out[k * P:(k + 1) * P, :])
                    tiles[k] = t
                return tiles

            def emit_ag_cols(ci, pair, lo, hi, tiles, spread):
                # partial AllGather: one head pair of chunk `ci`, columns
                # [lo, hi).  The last chunk's pair B splits in two so the
                # final gather carries half the data and fires earlier.
                c0 = ci * CHUNK
                w = hi - lo
                bounce_in = dram.tile([P, w], BF16, tag=f"binh{pair}",
                                      name=f"binh{ci}_{pair}_{lo}", bufs=2)
                bounce_out = dram.tile([grp * P, w], BF16,
                                       tag=f"bouth{pair}",
                                       name=f"bouth{ci}_{pair}_{lo}", bufs=2)
                # bounce rides gpsimd right before its trigger (sync may
                # still hold earlier gated tile loads)
                nc.gpsimd.dma_start(bounce_in[:],
                                    ot_sb[pair][:, c0 + lo:c0 + hi])
                nc.gpsimd.collective_compute(
                    "AllGather", mybir.AluOpType.bypass,
                    replica_groups=groups,
                    ins=[bounce_in.opt()], outs=[bounce_out.opt()])
                for r in range(grp):
                    k = 2 * r + pair
                    if k not in tiles:
                        tiles[k] = sb.tile([P, CHUNK], BF16, tag=f"ag{k}",
                                           name=f"ag{ci}_{k}", bufs=3)
                    # only the very last gather's loads spread over all
                    # queues (nothing left to block); earlier gated loads
                    # must stay on sync
                    eng = dma_engines[r % 3] if spread else nc.sync
                    eng.dma_start(tiles[k][:, lo:hi],
                                  bounce_out[r * P:(r + 1) * P, :])

            # ---- minimal upfront set: just what chunk-0's pass A needs so
            # the exp pipeline (the span's rate limiter) starts as early as
            # the input DMAs allow.  Everything else drains through the
            # work queue into PE idle slots.
            emit_qt(0, 0)
            emit_kt(0, 0)
            for jt in range(min(2, jpc)):
                emit_v(jt)
            for jt in range(min(2, jpc), jpc):
                work_early.append(lambda jt=jt: emit_v(jt))
            work_early.append(lambda: emit_qt(1, 0))
            work_early.append(lambda: emit_kt(1, 0))
            if nch > 1:
                for pair in range(2):
                    work_early.append(
                        lambda pair=pair: emit_qt(pair, 1))
            for ch in range(2, nch):
                for pair in range(2):
                    work_early.append(
                        lambda pair=pair, ch=ch: emit_qt(pair, ch))

            # ---- attention chunks ----
            last_parts = {}
            for ci in range(nch):
                jt_end = jpc * (ci + 1)
                c0 = ci * CHUNK
                last = ci == nch - 1

                if ci + 1 < nch:
                    for pair in range(2):
                        work_early.append(
                            lambda pair=pair, ch=ci + 1: emit_kt(pair, ch))
                    for jt in range(jpc * (ci + 1), jpc * (ci + 2)):
                        work_early.append(lambda jt=jt: emit_v(jt))

                binf = None if last else dram.tile(
                    [2 * P, CHUNK], BF16, tag="binf", name=f"binf{ci}",
                    bufs=2)

                def normalize(hpass, ot_ps, lo, hi):
                    # rcp of the sums row (both heads fused, columns
                    # [lo,hi)), gpsimd partition-broadcast, then one
                    # psum-read mul per head writes normalized O^T to SBUF.
                    # Everything stays off ACT: exp thruput is the pipeline
                    # rate limiter.
                    w = hi - lo
                    sr2 = sb.tile([1, 2 * w], F32, tag="sr",
                                  name=f"sr{ci}_{hpass}_{lo}", bufs=2)
                    rcp2 = sb.tile([1, 2 * w], F32, tag="rcp",
                                   name=f"rcp{ci}_{hpass}_{lo}", bufs=2)
                    nc.vector.tensor_copy(sr2[:, 0:w],
                                          ot_ps[0][DH:DHA, lo:hi])
                    nc.vector.tensor_copy(sr2[:, w:2 * w],
                                          ot_ps[1][DH:DHA, lo:hi])
                    nc.vector.reciprocal_approx_fast(rcp2[:], sr2[:])
                    bc2 = sb.tile([DH, 2 * w], F32, tag="bc",
                                  name=f"bc{ci}_{hpass}_{lo}", bufs=2)
                    nc.gpsimd.partition_broadcast(bc2[:], rcp2[:],
                                                  channels=DH)
                    for h2 in range(2):
                        nc.vector.tensor_mul(
                            ot_sb[hpass][h2 * DH:(h2 + 1) * DH,
                                         c0 + lo:c0 + hi],
                            ot_ps[h2][0:DH, lo:hi],
                            bc2[:, h2 * w:(h2 + 1) * w])

                HC = CHUNK // 2
                for hpass in range(2):
                    # heads 2*hpass, 2*hpass+1  (== head pair `hpass`)
                    ot_ps = [ps.tile([DHA, CHUNK], F32, tag=f"ot{h2}",
                                     name=f"ot{ci}_{hpass}_{h2}", bufs=1)
                             for h2 in range(2)]

                    def emit_st(jt):
                        # S^T tile = K_h @ Q_h^T (row-tiled, K=64) + exp
                        # for both heads + the diagonal band mask
                        rel = max(0, (jt - jpc * ci)) * P
                        s2 = ps.tile([P, 2 * CHUNK], F32, tag="s2",
                                     name=f"s{ci}_{hpass}_{jt}", bufs=2)
                        es = sb.tile([P, 2 * CHUNK], BF16, tag="es",
                                     name=f"es{ci}_{hpass}_{jt}", bufs=3)
                        for h2 in range(2):
                            nc.tensor.matmul(
                                s2[:, h2 * CHUNK + rel:(h2 + 1) * CHUNK],
                                lhsT=kt_sb[hpass][h2 * DH:(h2 + 1) * DH,
                                                  jt * P:(jt + 1) * P],
                                rhs=qt_sb[hpass][h2 * DH:(h2 + 1) * DH,
                                                 c0 + rel:c0 + CHUNK],
                                start=True, stop=True,
                                tile_position=(h2 * DH, 0))
                        nc.scalar.activation(
                            es.rearrange("p (t c) -> p t c", t=2)[:, :, rel:],
                            s2.rearrange("p (t c) -> p t c", t=2)[:, :, rel:],
                            ActFn.Exp)
                        if jt >= jpc * ci:
                            nc.vector.tensor_mul(
                                es.rearrange("p (t c) -> p t c",
                                             t=2)[:, :, rel:rel + P],
                                es.rearrange("p (t c) -> p t c",
                                             t=2)[:, :, rel:rel + P],
                                mask_sb.rearrange(
                                    "p (o c) -> p o c",
                                    o=1).broadcast_to((P, 2, P)))
                        return es

                    # software pipeline: S^T(jt+1) and fill work are queued
                    # between S^T(jt) and the exp-gated PV(jt) so the
                    # in-order PE queue never blocks on ACT latency
                    es_cur = emit_st(0)
                    for jt in range(jt_end):
                        rel = max(0, (jt - jpc * ci)) * P
                        es_nxt = emit_st(jt + 1) if jt + 1 < jt_end else None
                        # reservoir: drain gated projections only when their
                        # AllGather is surely complete; keep items back for
                        # the final AllGather's flight window
                        pop_work(late_ok=(hpass == 1 or last),
                                 late_floor=(3 if hpass == 1 else
                                             (3 if last else 4)))
                        for h2 in range(2):
                            h = 2 * hpass + h2
                            # O^T(+sums) accumulation: V_aug^T @ expS^T
                            nc.tensor.matmul(
                                ot_ps[h2][:, rel:CHUNK],
                                lhsT=v_sb[jt][:, h * DHA:(h + 1) * DHA],
                                rhs=es_cur[:, h2 * CHUNK + rel:
                                           (h2 + 1) * CHUNK],
                                start=(jt == 0), stop=(jt == jt_end - 1))
                        es_cur = es_nxt

                    normalize(hpass, ot_ps, 0, CHUNK)
                    if last:
                        emit_ag_cols(ci, hpass, 0, CHUNK, last_parts,
                                     spread=(hpass == 1))
                    else:
                        # bounce rides gpsimd right behind this pass's
                        # broadcast: on sync it would queue behind the
                        # previous chunk's gated ag loads
                        nc.gpsimd.dma_start(
                            binf[hpass * P:(hpass + 1) * P, :],
                            ot_sb[hpass][:, c0:c0 + CHUNK])

                if last:
                    agt = dict(last_parts)
                    last_parts = {}
                else:
                    agt = emit_ag_full(ci, binf)
                evens = [(k, agt[k], 0) for k in range(0, nko, 2)]
                odds = [(k, agt[k], 0) for k in range(1, nko, 2)]
                nm = HS // P
                # schedule-time floors: the static scheduler backfills PE
                # idle slots with any ready-by-its-model work — and its
                # AllGather estimate is optimistic.  A slug AG (25µs+) then
                # head-of-line-blocks the whole PE queue on a gated
                # projection.  Pin gated work past the worst-case landing.
                ag_eta = {0: 0.100, 1: 0.110, 2: 0.125}
                if not last:
                    for m in range(nm):
                        def run_proj(ci=ci, m=m, s=evens + odds):
                            with tc.tile_wait_until(ag_eta[ci]):
                                emit_proj(ci, m, s)
                        work_late.append(run_proj)
                else:
                    # three-stage projection for the last chunk: evens
                    # (pair-A AG, long landed), odd-left (first half-gather)
                    # and odd-right (final half-gather) — each stage's items
                    # fill the next gather's flight window; psum held across
                    # stages (no other psum users remain at the tail)
                    op_tiles = {}

                    def proj_ev(ci, m):
                        with tc.tile_wait_until(0.135):
                            op_tiles[m] = emit_proj(ci, m, evens, evac=False)

                    def proj_od(ci, m):
                        with tc.tile_wait_until(0.150):
                            emit_proj(ci, m, odds, op_ps=op_tiles.pop(m))

                    for fn in (proj_ev, proj_od):
                        for m in range(nm):
                            work_late.append(
                                lambda ci=ci, m=m, fn=fn: fn(ci, m))

            # tail: reservoir drains now — the reserved projections and the
            # pair-A halves fill the final AllGather's flight window before
            # the gated pair-B halves run
            while work_early or work_late:
                pop_work(late_ok=True, late_floor=0)

    if compile:
        nc.compile()
    return nc


def _pack(a, p=P):
    # [n*p, c] -> [p, n*c] partition-major repack
    n = a.shape[0] // p
    return np.ascontiguousarray(
        a.reshape(n, p, a.shape[1]).transpose(1, 0, 2).reshape(p, -1))


def make_in_maps(x, Wq, Wk, Wv, Wo, n_cores=N_CORES):
    import ml_dtypes
    bf16 = ml_dtypes.bfloat16
    scale = np.float32(DH ** -0.5)
    # band mask for the diagonal j-tile of S^T [j,i]: keep j <= i
    mask_b = np.triu(np.ones((P, P), np.float32)).astype(bf16)
    in_maps = []
    for c in range(n_cores):
        b, r = divmod(c, 4)
        hs = r * HS
        in_maps.append({
            "xT": _pack(np.ascontiguousarray(x[b].T)).astype(bf16),
            "wq": _pack(Wq[:, hs:hs + HS] * scale).astype(bf16),
            "wk": _pack(Wk[:, hs:hs + HS]).astype(bf16),
            "wv": _pack(Wv[:, hs:hs + HS]).astype(bf16),
            "wo": _pack(Wo[:, hs:hs + HS]).astype(bf16),
            "mask_c": mask_b,
        })
    return in_maps


def assemble_out(results, B, seq, n_cores=N_CORES):
    out = np.empty((B, seq, INNER), np.float32)
    for c in range(n_cores):
        b, r = divmod(c, 4)
        out[b][:, r * HS:(r + 1) * HS] = results[c]["outT"].T.astype(np.float32)
    return out


_NC_CACHE = {}


def kernel(x, Wq, Wk, Wv, Wo):
    from concourse import bass_utils
    x = np.asarray(x, np.float32)
    B, seq, dim = x.shape
    key = (seq, dim)
    if key not in _NC_CACHE:
        _NC_CACHE[key] = build_nc(seq=seq, dim=dim)
    nc = _NC_CACHE[key]
    in_maps = make_in_maps(x, np.asarray(Wq, np.float32),
                           np.asarray(Wk, np.float32),
                           np.asarray(Wv, np.float32),
                           np.asarray(Wo, np.float32))
    res = bass_utils.run_bass_kernel_spmd(
        nc, in_maps, core_ids=list(range(N_CORES)))
    return assemble_out(res.results, B, seq)
